# revision 1
# baseline (speedup 1.0000x reference)
"""GPT decoder (V=32000,S=1024,D=768,H=12,HID=3072,L=4,B=2) on 8 trn2 cores.

Sharding: sequence-parallel body — core c owns tokens [256c, 256c+256) of the
flattened [2048] token stream (cores 0-3 = batch 0, cores 4-7 = batch 1).
Per layer, each core computes qkv for its tokens, K/V are exchanged with an
AllGather inside each 4-core batch group, attention/FFN stay local.  The tied
lm_head runs per-core over the full vocab for the local 256 tokens.
Matmuls in bf16 with fp32 PSUM accumulation; activations/norms in fp32.
Activations are feature-major [D, tok] so the contraction dim is on partitions.
"""
import math

import ml_dtypes
import numpy as np

import concourse.bass as bass
import concourse.mybir as mybir
import concourse.tile as tile
from concourse import bacc
from concourse.bass_utils import run_bass_kernel_spmd

F32 = mybir.dt.float32
BF16 = mybir.dt.bfloat16
AF = mybir.ActivationFunctionType
ALU = mybir.AluOpType

N_CORES = 8
GROUPS = [[0, 1, 2, 3], [4, 5, 6, 7]]
V, S, D, H, HID, L, B = 32000, 1024, 768, 12, 3072, 4, 2
HD = D // H          # 64
TOK = 256            # tokens per core
NK = D // 128        # 6 feature chunks
NM_QKV = 3 * D // 128   # 18
NM_HID = HID // 128     # 24
EPS = 1e-5
VB = 500             # lm_head vocab block
NVB = V // VB        # 64

TRACE = False
LAST_RESULT = None

_NC_CACHE = None


def _ln(nc, tc, pools, x_fm, g_ap, b_ap, out_bf):
    """LayerNorm over features (partition dim) via ones-matmul reductions.

    x_fm: [128, NK, 256] f32 sbuf.  g_ap/b_ap: [128,1] per-chunk slices fn.
    out_bf: [128, NK, 256] bf16 sbuf tile to fill with gamma*x_hat+beta.
    """
    ps_stat, ps_mm, tmp, stt = pools["ps_stat"], pools["ps_mm"], pools["tmp"], pools["stt"]
    ones_bf = pools["ones_bf"]
    ones_row = pools["ones_row"]

    s1 = ps_stat.tile([1, TOK], F32, tag="lnstat")
    s2 = ps_stat.tile([1, TOK], F32, tag="lnstat")
    for k in range(NK):
        xb = tmp.tile([128, TOK], BF16, tag="lnxb")
        nc.vector.tensor_copy(xb[:], x_fm[:, k, :])
        nc.tensor.matmul(s1[:], ones_bf[:], xb[:], start=(k == 0), stop=(k == NK - 1))
        sq = tmp.tile([128, TOK], BF16, tag="lnsq")
        nc.vector.tensor_mul(sq[:], xb[:], xb[:])
        nc.tensor.matmul(s2[:], ones_bf[:], sq[:], start=(k == 0), stop=(k == NK - 1))

    mean = stt.tile([1, TOK], F32, tag="mean")
    nc.vector.tensor_scalar_mul(mean[:], s1[:], 1.0 / D)
    var = stt.tile([1, TOK], F32, tag="var")
    nc.vector.tensor_scalar_mul(var[:], s2[:], 1.0 / D)
    msq = stt.tile([1, TOK], F32, tag="msq")
    nc.vector.tensor_mul(msq[:], mean[:], mean[:])
    nc.vector.tensor_sub(var[:], var[:], msq[:])
    nc.vector.tensor_scalar_add(var[:], var[:], EPS)
    rec = stt.tile([1, TOK], F32, tag="rec")
    nc.vector.reciprocal(rec[:], var[:])
    a = stt.tile([1, TOK], F32, tag="a")
    nc.scalar.sqrt(a[:], rec[:])                      # rstd = sqrt(1/(var+eps))
    colb = stt.tile([1, TOK], F32, tag="colb")
    nc.vector.tensor_mul(colb[:], mean[:], a[:])
    nc.vector.tensor_scalar_mul(colb[:], colb[:], -1.0)  # -mean*rstd

    ba = ps_mm.tile([128, TOK], F32, tag="mm")
    nc.tensor.matmul(ba[:], ones_row[:], a[:], start=True, stop=True)
    bb = ps_mm.tile([128, TOK], F32, tag="mm")
    nc.tensor.matmul(bb[:], ones_row[:], colb[:], start=True, stop=True)

    for k in range(NK):
        t = tmp.tile([128, TOK], F32, tag="lnt")
        nc.vector.tensor_mul(t[:], x_fm[:, k, :], ba[:])
        nc.vector.tensor_add(t[:], t[:], bb[:])
        nc.scalar.activation(out_bf[:, k, :], t[:], AF.Identity,
                             bias=b_ap[k], scale=g_ap[k])


def build_nc():
    nc = bacc.Bacc("TRN2", target_bir_lowering=False, debug=False,
                   enable_asserts=True, num_devices=N_CORES)

    d_x0 = nc.dram_tensor("x0", [NK, 128, TOK], F32, kind="ExternalInput")
    d_mask = nc.dram_tensor("mask", [8, 128, TOK], F32, kind="ExternalInput")
    d_qkvw = nc.dram_tensor("qkvw", [L, NK, 128, 3 * D], BF16, kind="ExternalInput")
    d_qkvb = nc.dram_tensor("qkvb", [L, 128, NM_QKV], F32, kind="ExternalInput")
    d_projw = nc.dram_tensor("projw", [L, NK, 128, D], BF16, kind="ExternalInput")
    d_projb = nc.dram_tensor("projb", [L, 128, NK], F32, kind="ExternalInput")
    d_f1w = nc.dram_tensor("f1w", [L, NK, 128, HID], BF16, kind="ExternalInput")
    d_f1b = nc.dram_tensor("f1b", [L, 128, NM_HID], F32, kind="ExternalInput")
    d_f2w = nc.dram_tensor("f2w", [L, NM_HID, 128, D], BF16, kind="ExternalInput")
    d_f2b = nc.dram_tensor("f2b", [L, 128, NK], F32, kind="ExternalInput")
    d_n1g = nc.dram_tensor("n1g", [L, 128, NK], F32, kind="ExternalInput")
    d_n1b = nc.dram_tensor("n1b", [L, 128, NK], F32, kind="ExternalInput")
    d_n2g = nc.dram_tensor("n2g", [L, 128, NK], F32, kind="ExternalInput")
    d_n2b = nc.dram_tensor("n2b", [L, 128, NK], F32, kind="ExternalInput")
    d_fing = nc.dram_tensor("fing", [128, NK], F32, kind="ExternalInput")
    d_finb = nc.dram_tensor("finb", [128, NK], F32, kind="ExternalInput")
    d_wemb = nc.dram_tensor("wemb", [NVB, NK, 128, VB], BF16, kind="ExternalInput")
    d_out = nc.dram_tensor("logits", [TOK, V], F32, kind="ExternalOutput")

    with tile.TileContext(nc) as tc:
        from contextlib import ExitStack
        with ExitStack() as ctx:
            const = ctx.enter_context(tc.tile_pool(name="const", bufs=1))
            res = ctx.enter_context(tc.tile_pool(name="res", bufs=1))
            tmp = ctx.enter_context(tc.tile_pool(name="tmp", bufs=3))
            stt = ctx.enter_context(tc.tile_pool(name="stt", bufs=2))
            attn = ctx.enter_context(tc.tile_pool(name="attn", bufs=2))
            lmo = ctx.enter_context(tc.tile_pool(name="lmo", bufs=3))
            ps_mm = ctx.enter_context(tc.tile_pool(name="ps_mm", bufs=2, space="PSUM"))
            ps_sc = ctx.enter_context(tc.tile_pool(name="ps_sc", bufs=2, space="PSUM"))
            ps_tr = ctx.enter_context(tc.tile_pool(name="ps_tr", bufs=2, space="PSUM"))
            ps_stat = ctx.enter_context(tc.tile_pool(name="ps_stat", bufs=2, space="PSUM"))
            dram = ctx.enter_context(tc.tile_pool(name="dram", bufs=2, space="DRAM"))

            identity = const.tile([128, 128], BF16)
            from concourse.masks import make_identity
            make_identity(nc, identity[:])
            ones_bf = const.tile([128, 1], BF16)
            nc.any.memset(ones_bf[:], 1.0)
            ones_row = const.tile([1, 128], F32)
            nc.any.memset(ones_row[:], 1.0)
            ones2 = const.tile([128, 128], F32)
            nc.any.memset(ones2[:], 1.0)

            # Residual stream + mask + params, resident in SBUF
            x_fm = res.tile([128, NK, TOK], F32)
            for k in range(NK):
                nc.sync.dma_start(x_fm[:, k, :], d_x0.ap()[k])
            mask_t = res.tile([128, 8, TOK], F32)
            for t in range(8):
                nc.sync.dma_start(mask_t[:, t, :], d_mask.ap()[t])
            qkvb_a = res.tile([128, L, NM_QKV], F32)
            projb_a = res.tile([128, L, NK], F32)
            f1b_a = res.tile([128, L, NM_HID], F32)
            f2b_a = res.tile([128, L, NK], F32)
            n1g_a = res.tile([128, L, NK], F32)
            n1b_a = res.tile([128, L, NK], F32)
            n2g_a = res.tile([128, L, NK], F32)
            n2b_a = res.tile([128, L, NK], F32)
            fing_a = res.tile([128, NK], F32)
            finb_a = res.tile([128, NK], F32)
            for l in range(L):
                nc.sync.dma_start(qkvb_a[:, l, :], d_qkvb.ap()[l])
                nc.sync.dma_start(projb_a[:, l, :], d_projb.ap()[l])
                nc.sync.dma_start(f1b_a[:, l, :], d_f1b.ap()[l])
                nc.sync.dma_start(f2b_a[:, l, :], d_f2b.ap()[l])
                nc.sync.dma_start(n1g_a[:, l, :], d_n1g.ap()[l])
                nc.sync.dma_start(n1b_a[:, l, :], d_n1b.ap()[l])
                nc.sync.dma_start(n2g_a[:, l, :], d_n2g.ap()[l])
                nc.sync.dma_start(n2b_a[:, l, :], d_n2b.ap()[l])
            nc.sync.dma_start(fing_a[:], d_fing.ap())
            nc.sync.dma_start(finb_a[:], d_finb.ap())

            pools = dict(ps_stat=ps_stat, ps_mm=ps_mm, tmp=tmp, stt=stt,
                         ones_bf=ones_bf, ones_row=ones_row)

            h_bf = res.tile([128, NK, TOK], BF16)
            qkv_sb = res.tile([128, NM_QKV, TOK], BF16)
            # v_own: per head 66 cols = [onesA | v(64) | onesB]; even heads use
            # cols [1:66] (ones last -> den row 64), odd heads cols [0:65]
            # (ones first -> den row 63) so AV lands vals at the head's rows.
            v_own = res.tile([128, 2, H, 66], BF16)
            nc.any.memset(v_own[:, :, :, 0:1], 1.0)
            nc.any.memset(v_own[:, :, :, 65:66], 1.0)
            k_all = res.tile([128, NK, 4 * TOK], BF16)
            v_all = res.tile([128, 8, H * 66], BF16)
            vals_fm = res.tile([128, NK, TOK], BF16)
            h2_sb = res.tile([128, NM_HID, TOK], BF16)

            def qslice(h, qt):
                return qkv_sb[(h % 2) * 64:(h % 2) * 64 + 64, h // 2,
                              128 * qt:128 * qt + 128]

            for l in range(L):
                gs = [n1g_a[:, l, k:k + 1] for k in range(NK)]
                bs = [n1b_a[:, l, k:k + 1] for k in range(NK)]
                _ln(nc, tc, pools, x_fm, gs, bs, h_bf)

                # qkv
                with tc.tile_pool(name="wqkv", bufs=NK) as pw:
                    wk = []
                    for k in range(NK):
                        w = pw.tile([128, 3 * D], BF16, tag="w")
                        nc.sync.dma_start(w[:], d_qkvw.ap()[l, k])
                        wk.append(w)
                    for m in range(NM_QKV):
                        ps = ps_mm.tile([128, TOK], F32, tag="mm")
                        for k in range(NK):
                            nc.tensor.matmul(ps[:], wk[k][:, 128 * m:128 * (m + 1)],
                                             h_bf[:, k, :],
                                             start=(k == 0), stop=(k == NK - 1))
                        scale = 1.0 / math.sqrt(HD) if m < NK else 1.0
                        nc.scalar.activation(qkv_sb[:, m, :], ps[:], AF.Identity,
                                             bias=qkvb_a[:, l, m:m + 1], scale=scale)

                # own-chunk v -> token-major
                for h in range(H):
                    o = (h % 2) * 64
                    for t in range(2):
                        src = qkv_sb[o:o + 64, 12 + h // 2, 128 * t:128 * (t + 1)]
                        pt = ps_tr.tile([128, 64], BF16, tag="tr")
                        nc.tensor.transpose(pt[:], src,
                                            identity[o:o + 64, o:o + 64])
                        nc.vector.tensor_copy(v_own[:, t, h, 1:65], pt[:])

                # KV exchange within batch group: slots 0-5 = k chunks (256 of
                # 264 cols), slots 6-11 = v_own (2 tok-chunks x 3 blocks of
                # 4 heads x 66).
                b_in = dram.tile([12, 128, 264], BF16, tag="bin")
                b_out = dram.tile([48, 128, 264], BF16, tag="bout")
                for k in range(NK):
                    nc.sync.dma_start(b_in[k, :, 0:TOK], qkv_sb[:, NK + k, :])
                for t in range(2):
                    for j in range(3):
                        nc.sync.dma_start(b_in[6 + 3 * t + j],
                                          v_own[:, t, 4 * j:4 * (j + 1), :])
                nc.gpsimd.collective_compute(
                    "AllGather", ALU.bypass, replica_groups=GROUPS,
                    ins=[b_in.opt()], outs=[b_out.opt()])
                for c in range(4):
                    for k in range(NK):
                        nc.sync.dma_start(k_all[:, k, TOK * c:TOK * (c + 1)],
                                          b_out[12 * c + k, :, 0:TOK])
                    for t in range(2):
                        for j in range(3):
                            nc.sync.dma_start(
                                v_all[:, 2 * c + t, 264 * j:264 * (j + 1)],
                                b_out[12 * c + 6 + 3 * t + j])

                # attention: S^T per kt-chunk, exp, AV with ones-col -> den row
                for h in range(H):
                    o = (h % 2) * 64
                    kslc = slice(o, o + 64)
                    av = ps_mm.tile([128, TOK], F32, tag="mm")
                    dn = ps_stat.tile([1, TOK], F32, tag="lnstat")
                    vcol = 66 * h + 1
                    for kc in range(8):
                        st = ps_sc.tile([128, TOK], F32, tag="sc")
                        nc.tensor.matmul(
                            st[:],
                            k_all[kslc, h // 2, 128 * kc:128 * (kc + 1)],
                            qkv_sb[kslc, h // 2, :],
                            start=True, stop=True)
                        nc.vector.tensor_add(st[:], st[:], mask_t[:, kc, :])
                        pt_t = attn.tile([128, TOK], BF16, tag="ptsb")
                        nc.scalar.activation(pt_t[:], st[:], AF.Exp)
                        nc.tensor.matmul(av[o:o + 64, :],
                                         v_all[:, kc, vcol:vcol + 64],
                                         pt_t[:],
                                         start=(kc == 0), stop=(kc == 7))
                        nc.tensor.matmul(dn[:], ones_bf[:], pt_t[:],
                                         start=(kc == 0), stop=(kc == 7))
                    rden = stt.tile([1, TOK], F32, tag="rden")
                    nc.vector.reciprocal(rden[:], dn[:])
                    bc = ps_tr.tile([128, TOK], F32, tag="tr")
                    nc.tensor.matmul(bc[o:o + 64, :], ones2[0:1, 0:64],
                                     rden[:], start=True, stop=True)
                    bcs = tmp.tile([128, TOK], F32, tag="lnt")
                    nc.scalar.copy(bcs[o:o + 64, :], bc[o:o + 64, :])
                    nc.vector.tensor_mul(vals_fm[o:o + 64, h // 2, :],
                                         av[o:o + 64, :], bcs[o:o + 64, :])

                # proj + residual
                with tc.tile_pool(name="wproj", bufs=NK) as pw:
                    pk = []
                    for k in range(NK):
                        w = pw.tile([128, D], BF16, tag="w")
                        nc.sync.dma_start(w[:], d_projw.ap()[l, k])
                        pk.append(w)
                    for m in range(NK):
                        ps = ps_mm.tile([128, TOK], F32, tag="mm")
                        for k in range(NK):
                            nc.tensor.matmul(ps[:], pk[k][:, 128 * m:128 * (m + 1)],
                                             vals_fm[:, k, :],
                                             start=(k == 0), stop=(k == NK - 1))
                        t = tmp.tile([128, TOK], F32, tag="lnt")
                        nc.scalar.activation(t[:], ps[:], AF.Identity,
                                             bias=projb_a[:, l, m:m + 1])
                        nc.vector.tensor_add(x_fm[:, m, :], x_fm[:, m, :], t[:])

                # LN2 + FFN
                gs = [n2g_a[:, l, k:k + 1] for k in range(NK)]
                bs = [n2b_a[:, l, k:k + 1] for k in range(NK)]
                _ln(nc, tc, pools, x_fm, gs, bs, h_bf)

                with tc.tile_pool(name="wf1", bufs=NK) as pw:
                    wf = []
                    for k in range(NK):
                        w = pw.tile([128, HID], BF16, tag="w")
                        nc.sync.dma_start(w[:], d_f1w.ap()[l, k])
                        wf.append(w)
                    for m in range(NM_HID):
                        ps = ps_mm.tile([128, TOK], F32, tag="mm")
                        for k in range(NK):
                            nc.tensor.matmul(ps[:], wf[k][:, 128 * m:128 * (m + 1)],
                                             h_bf[:, k, :],
                                             start=(k == 0), stop=(k == NK - 1))
                        nc.scalar.activation(h2_sb[:, m, :], ps[:], AF.Gelu,
                                             bias=f1b_a[:, l, m:m + 1])

                with tc.tile_pool(name="wf2", bufs=NM_HID) as pw:
                    wf = []
                    for k in range(NM_HID):
                        w = pw.tile([128, D], BF16, tag="w")
                        nc.sync.dma_start(w[:], d_f2w.ap()[l, k])
                        wf.append(w)
                    for m in range(NK):
                        ps = ps_mm.tile([128, TOK], F32, tag="mm")
                        for k in range(NM_HID):
                            nc.tensor.matmul(ps[:], wf[k][:, 128 * m:128 * (m + 1)],
                                             h2_sb[:, k, :],
                                             start=(k == 0), stop=(k == NM_HID - 1))
                        t = tmp.tile([128, TOK], F32, tag="lnt")
                        nc.scalar.activation(t[:], ps[:], AF.Identity,
                                             bias=f2b_a[:, l, m:m + 1])
                        nc.vector.tensor_add(x_fm[:, m, :], x_fm[:, m, :], t[:])

            # final LN + lm_head
            gs = [fing_a[:, k:k + 1] for k in range(NK)]
            bs = [finb_a[:, k:k + 1] for k in range(NK)]
            _ln(nc, tc, pools, x_fm, gs, bs, h_bf)

            with tc.tile_pool(name="wlm", bufs=12) as pw:
                for b in range(NVB):
                    wvs = []
                    for k in range(NK):
                        w = pw.tile([128, VB], BF16, tag="w")
                        nc.sync.dma_start(w[:], d_wemb.ap()[b, k])
                        wvs.append(w)
                    for qt in range(2):
                        ps = ps_sc.tile([128, VB], F32, tag="sc")
                        for k in range(NK):
                            nc.tensor.matmul(ps[:],
                                             h_bf[:, k, 128 * qt:128 * (qt + 1)],
                                             wvs[k][:],
                                             start=(k == 0), stop=(k == NK - 1))
                        ot = lmo.tile([128, VB], F32, tag="ot")
                        nc.vector.tensor_copy(ot[:], ps[:])
                        nc.sync.dma_start(
                            d_out.ap()[128 * qt:128 * (qt + 1), VB * b:VB * (b + 1)],
                            ot[:])

    nc.compile()
    return nc


def _prep_inputs(W_emb, pos_emb, norm1_g, norm1_b, qkv_w, qkv_b, proj_w, proj_b,
                 norm2_g, norm2_b, ffn_w1, ffn_b1, ffn_w2, ffn_b2, fin_g, fin_b,
                 input_ids):
    bf = ml_dtypes.bfloat16
    f32 = np.float32

    def tp(a):  # [L, out, in] -> [L, NK, 128, out] bf16
        a = np.asarray(a, f32)
        out_dim = a.shape[1]
        return np.ascontiguousarray(
            a.transpose(0, 2, 1).reshape(L, NK, 128, out_dim)).astype(bf)

    def btile(a, nm):  # [L, nm*128] -> [L, 128, nm]
        return np.ascontiguousarray(
            np.asarray(a, f32).reshape(L, nm, 128).transpose(0, 2, 1))

    qkv_r = np.asarray(qkv_w, f32).reshape(L, H, 3, HD, D).transpose(0, 2, 1, 3, 4) \
        .reshape(L, 3 * D, D)
    qkv_b_r = np.asarray(qkv_b, f32).reshape(L, H, 3, HD).transpose(0, 2, 1, 3) \
        .reshape(L, 3 * D).copy()
    qkv_b_r[:, :D] *= 1.0 / math.sqrt(HD)   # q bias shares the score scale

    f2w = np.asarray(ffn_w2, f32)  # [L, D, HID]
    f2w_t = np.ascontiguousarray(
        f2w.transpose(0, 2, 1).reshape(L, NM_HID, 128, D)).astype(bf)

    W_emb = np.asarray(W_emb, f32)
    wemb_t = np.ascontiguousarray(
        W_emb.T.reshape(NK, 128, NVB, VB).transpose(2, 0, 1, 3)).astype(bf)

    ids = np.asarray(input_ids).reshape(-1).astype(np.int64)
    x0 = W_emb[ids] * math.sqrt(D)
    x0 = x0 + np.asarray(pos_emb, f32)[np.tile(np.arange(S), B)]

    common = {
        "qkvw": tp(qkv_r), "qkvb": btile(qkv_b_r, NM_QKV),
        "projw": tp(np.asarray(proj_w, f32)), "projb": btile(proj_b, NK),
        "f1w": tp(np.asarray(ffn_w1, f32)), "f1b": btile(ffn_b1, NM_HID),
        "f2w": f2w_t, "f2b": btile(ffn_b2, NK),
        "n1g": btile(norm1_g, NK), "n1b": btile(norm1_b, NK),
        "n2g": btile(norm2_g, NK), "n2b": btile(norm2_b, NK),
        "fing": np.ascontiguousarray(np.asarray(fin_g, f32).reshape(NK, 128).T),
        "finb": np.ascontiguousarray(np.asarray(fin_b, f32).reshape(NK, 128).T),
        "wemb": wemb_t,
    }

    kg = np.arange(4 * TOK)
    in_maps = []
    for c in range(N_CORES):
        xs = np.ascontiguousarray(
            x0[TOK * c:TOK * (c + 1)].T.reshape(NK, 128, TOK)).astype(f32)
        p = c % 4
        qg = p * TOK + np.arange(TOK)
        m = np.where(qg[None, :] >= kg[:, None], 0.0, -1e9).astype(f32)
        m = np.ascontiguousarray(m.reshape(8, 128, TOK))
        in_maps.append({"x0": xs, "mask": m, **common})
    return in_maps


def kernel(**inputs):
    global LAST_RESULT, _NC_CACHE
    in_maps = _prep_inputs(**inputs)
    if _NC_CACHE is None:
        _NC_CACHE = build_nc()
    res = run_bass_kernel_spmd(_NC_CACHE, in_maps, list(range(N_CORES)),
                               trace=TRACE)
    LAST_RESULT = res
    logits = np.concatenate(
        [np.asarray(res.results[c]["logits"]) for c in range(N_CORES)], axis=0)
    return logits.reshape(B, S, V).astype(np.float32)



# revision 24
# speedup vs baseline: 1.4383x; 1.4383x over previous
"""GPT decoder (V=32000,S=1024,D=768,H=12,HID=3072,L=4,B=2) on 8 trn2 cores.

Sharding: sequence-parallel body -- core c owns tokens [256c, 256c+256) of the
flattened [2048] token stream (cores 0-3 = batch 0, cores 4-7 = batch 1).
Per layer K/V are exchanged with an AllGather inside each 4-core batch group;
attention/FFN stay local.  The tied lm_head is vocab-sharded: final hidden
states are AllGathered across all 8 cores and each core computes logits for
its 4000-vocab slice over all 2048 tokens (bf16 out, host concat + f32 cast).
Matmuls in bf16 with fp32 PSUM accumulation; activations/norms in fp32.
Activations are feature-major [D, tok] so the contraction dim is on partitions.
"""
import math
from contextlib import ExitStack

import ml_dtypes
import numpy as np

import concourse.bass as bass
import concourse.mybir as mybir
import concourse.tile as tile
from concourse import bacc
from concourse.bass_utils import run_bass_kernel_spmd
from concourse.masks import make_identity

F32 = mybir.dt.float32
BF16 = mybir.dt.bfloat16
AF = mybir.ActivationFunctionType
ALU = mybir.AluOpType

N_CORES = 8
GROUPS = [[0, 1, 2, 3], [4, 5, 6, 7]]
ALLG = [[0, 1, 2, 3, 4, 5, 6, 7]]
V, S, D, H, HID, L, B = 32000, 1024, 768, 12, 3072, 4, 2
HD = D // H          # 64
TOK = 256            # tokens per core
NK = D // 128        # 6 feature chunks
NM_QKV = 3 * D // 128   # 18
NM_HID = HID // 128     # 24
EPS = 1e-5
VC = V // N_CORES    # 4000 vocab per core
VB = 500
TOKALL = B * S       # 2048
NTC = TOKALL // 128  # 16 token chunks in lm_head

# allb packed param columns: per-layer base l*78
OFF_QKVB, OFF_PROJB, OFF_F1B, OFF_F2B = 0, 18, 24, 48
OFF_N1G, OFF_N1B, OFF_N2G, OFF_N2B = 54, 60, 66, 72
PER_L = 78
OFF_FING, OFF_FINB = L * PER_L, L * PER_L + 6
NALLB = L * PER_L + 12

TRACE = False
LAST_RESULT = None
_NC_CACHE = None
GELU_AF = AF.Gelu   # debug_sim swaps to Tanh (CoreSim lacks Gelu)


def _ln(nc, tc, pools, x_fm, g_aps, b_aps, out_bf):
    """LayerNorm over features (partition dim); out_bf = g*x_hat+b in bf16."""
    ps_a, tmp, stt = pools["ps_a"], pools["tmp"], pools["stt"]
    ones_bf, ones_row = pools["ones_bf"], pools["ones_row"]

    xb = tmp.tile([128, NK, TOK], BF16, tag="xb")
    nc.vector.tensor_copy(xb[:], x_fm[:])
    sq = tmp.tile([128, NK, TOK], BF16, tag="sq")
    nc.vector.tensor_mul(sq[:], xb[:], xb[:])
    s1 = ps_a.tile([1, TOK], F32, tag="b")
    s2 = ps_a.tile([1, TOK], F32, tag="b")
    for k in range(NK):
        nc.tensor.matmul(s1[:], ones_bf[:], xb[:, k, :], start=(k == 0), stop=(k == NK - 1))
        nc.tensor.matmul(s2[:], ones_bf[:], sq[:, k, :], start=(k == 0), stop=(k == NK - 1))

    m2 = stt.tile([1, TOK], F32, tag="m2")
    nc.vector.tensor_scalar(m2[:], s2[:], 1.0 / D, EPS, ALU.mult, ALU.add)
    s1s = stt.tile([1, TOK], F32, tag="s1s")
    nc.vector.tensor_copy(s1s[:], s1[:])
    t1 = stt.tile([1, TOK], F32, tag="t1")
    nc.vector.tensor_mul(t1[:], s1s[:], s1s[:])
    varp = stt.tile([1, TOK], F32, tag="varp")
    nc.vector.scalar_tensor_tensor(varp[:], t1[:], -1.0 / (D * D), m2[:], ALU.mult, ALU.add)
    rec = stt.tile([1, TOK], F32, tag="rec")
    nc.vector.reciprocal_approx_fast(rec[:], varp[:])
    a = stt.tile([1, TOK], F32, tag="a")
    nc.scalar.sqrt(a[:], rec[:])                     # rstd
    colb = stt.tile([1, TOK], F32, tag="colb")
    nc.vector.scalar_tensor_tensor(colb[:], s1[:], -1.0 / D, a[:], ALU.mult, ALU.mult)

    ba = ps_a.tile([128, TOK], F32, tag="mm")
    nc.tensor.matmul(ba[:], ones_row[:], a[:], start=True, stop=True)
    bb = ps_a.tile([128, TOK], F32, tag="mm")
    nc.tensor.matmul(bb[:], ones_row[:], colb[:], start=True, stop=True)

    t = tmp.tile([128, NK, TOK], F32, tag="lnt")
    ba_b = ba[:].unsqueeze(1).broadcast_to([128, NK, TOK])
    bb_b = bb[:].unsqueeze(1).broadcast_to([128, NK, TOK])
    nc.vector.tensor_mul(t[:], x_fm[:], ba_b)
    nc.vector.tensor_add(t[:], t[:], bb_b)
    for k in range(NK):
        nc.scalar.activation(out_bf[:, k, :], t[:, k, :], AF.Identity,
                             bias=b_aps[k], scale=g_aps[k])


def build_nc():
    nc = bacc.Bacc("TRN2", target_bir_lowering=False, debug=False,
                   enable_asserts=True, num_devices=N_CORES)

    d_x0 = nc.dram_tensor("x0", [128, NK, TOK], F32, kind="ExternalInput")
    d_mask = nc.dram_tensor("mask01", [128, 8, TOK], BF16, kind="ExternalInput")
    d_allb = nc.dram_tensor("allb", [128, NALLB], F32, kind="ExternalInput")
    d_qkvw = nc.dram_tensor("qkvw", [L, 128, NK, 3 * D], BF16, kind="ExternalInput")
    d_projw = nc.dram_tensor("projw", [L, 128, NK, D], BF16, kind="ExternalInput")
    d_f1w = nc.dram_tensor("f1w", [L, 4, 128, 6, NK, 128], BF16, kind="ExternalInput")
    d_f2w = nc.dram_tensor("f2w", [L, 2, 128, 12, D], BF16, kind="ExternalInput")
    d_wemb = nc.dram_tensor("wemb", [2, 128, 4, NK, VB], BF16, kind="ExternalInput")
    d_out = nc.dram_tensor("logits", [TOKALL, VC], BF16, kind="ExternalOutput")

    with tile.TileContext(nc) as tc:
        with ExitStack() as ctx:
            const = ctx.enter_context(tc.tile_pool(name="const", bufs=1))
            res = ctx.enter_context(tc.tile_pool(name="res", bufs=1))
            tmp = ctx.enter_context(tc.tile_pool(name="tmp", bufs=1))
            stt = ctx.enter_context(tc.tile_pool(name="stt", bufs=1))
            stt2 = ctx.enter_context(tc.tile_pool(name="stt2", bufs=2))
            pt_pool = ctx.enter_context(tc.tile_pool(name="pt", bufs=6))
            dram = ctx.enter_context(tc.tile_pool(name="dram", bufs=2, space="DRAM"))

            identity = const.tile([128, 128], BF16)
            make_identity(nc, identity[:])
            ones_bf = const.tile([128, 1], BF16)
            nc.any.memset(ones_bf[:], 1.0)
            ones_row = const.tile([1, 128], F32)
            nc.any.memset(ones_row[:], 1.0)
            ones_r64 = const.tile([1, 64], BF16)
            nc.any.memset(ones_r64[:], 1.0)
            zpad = const.tile([128, NK, 8], BF16)
            nc.any.memset(zpad[:], 0.0)

            # Residual stream + mask + params, resident in SBUF (one DMA each)
            x_fm = res.tile([128, NK, TOK], F32)
            nc.sync.dma_start(x_fm[:], d_x0.ap())
            mask01 = res.tile([128, 8, TOK], BF16)
            nc.sync.dma_start(mask01[:], d_mask.ap())
            allb = res.tile([128, NALLB], F32)
            nc.sync.dma_start(allb[:], d_allb.ap())

            def bcol(off, j):
                return allb[:, off + j:off + j + 1]

            h_bf = res.tile([128, NK, TOK], BF16)
            qkv_sb = res.tile([128, NM_QKV, TOK], BF16)
            # v_own: per head 66 cols = [onesA | v(64) | onesB]; even heads use
            # cols [1:66] (ones last -> den row 64), odd heads cols [0:65]
            # (ones first -> den row 63).
            v_own = res.tile([128, 2, H, 66], BF16)
            nc.any.memset(v_own[:, :, :, 0:1], 1.0)
            nc.any.memset(v_own[:, :, :, 65:66], 1.0)
            krank = [res.tile([128, NK, TOK], BF16, tag=f"kr{c}", name=f"krank{c}")
                     for c in range(4)]
            vrank = [res.tile([128, 2, H * 66], BF16, tag=f"vr{c}", name=f"vrank{c}")
                     for c in range(4)]
            vals_fm = res.tile([128, NK, TOK], BF16)
            h2_sb = res.tile([128, NM_HID, TOK], BF16)
            f2acc = res.tile([128, NK, TOK], BF16)

            with ExitStack() as bctx:
                wqkv_p = bctx.enter_context(tc.tile_pool(name="wqkv", bufs=1))
                wproj_p = bctx.enter_context(tc.tile_pool(name="wproj", bufs=1))
                wf1_p = bctx.enter_context(tc.tile_pool(name="wf1", bufs=2))
                wf2_p = bctx.enter_context(tc.tile_pool(name="wf2", bufs=2))
                # PSUM banks: ps_a tag "mm" 2 + tag "b" 2, ps_st tag "st" 3 = 7
                ps_a = bctx.enter_context(tc.tile_pool(name="ps_a", bufs=2, space="PSUM"))
                ps_st = bctx.enter_context(tc.tile_pool(name="ps_st", bufs=3, space="PSUM"))

                pools = dict(ps_a=ps_a, tmp=tmp, stt=stt,
                             ones_bf=ones_bf, ones_row=ones_row)

                wqkv = wqkv_p.tile([128, NK, 3 * D], BF16, tag="w")
                wproj = wproj_p.tile([128, NK, D], BF16, tag="w")
                nc.sync.dma_start(wqkv[:], d_qkvw.ap()[0])
                nc.sync.dma_start(wproj[:], d_projw.ap()[0])

                for l in range(L):
                    gs = [bcol(l * PER_L + OFF_N1G, k) for k in range(NK)]
                    bs = [bcol(l * PER_L + OFF_N1B, k) for k in range(NK)]
                    _ln(nc, tc, pools, x_fm, gs, bs, h_bf)

                    # qkv: K,V rows first so the collective can start early
                    for m in list(range(NK, NM_QKV)) + list(range(NK)):
                        ps = ps_a.tile([128, TOK], F32, tag="mm")
                        for k in range(NK):
                            nc.tensor.matmul(ps[:], wqkv[:, k, 128 * m:128 * (m + 1)],
                                             h_bf[:, k, :],
                                             start=(k == 0), stop=(k == NK - 1))
                        nc.vector.tensor_scalar_add(
                            qkv_sb[:, m, :], ps[:], bcol(l * PER_L + OFF_QKVB, m))
                        if m == NM_QKV - 1:
                            # own-chunk v -> token-major, then stage + gather
                            for h in range(H):
                                o = (h % 2) * 64
                                for t in range(2):
                                    src = qkv_sb[o:o + 64, 2 * NK + h // 2,
                                                 128 * t:128 * (t + 1)]
                                    pv = ps_a.tile([128, 64], BF16, tag="b")
                                    nc.tensor.transpose(pv[:], src,
                                                        identity[o:o + 64, o:o + 64])
                                    nc.vector.tensor_copy(v_own[:, t, h, 1:65], pv[:])
                            b_in = dram.tile([12, 128, 264], BF16, tag="bin")
                            b_out = dram.tile([48, 128, 264], BF16, tag="bout")
                            nc.sync.dma_start(
                                b_in[0:6, :, 0:TOK].transpose([1, 0, 2]),
                                qkv_sb[:, NK:2 * NK, :])
                            nc.sync.dma_start(
                                b_in[0:6, :, TOK:264].transpose([1, 0, 2]),
                                zpad[:])
                            nc.sync.dma_start(
                                b_in[6:12, :, :].transpose([1, 0, 2]),
                                v_own[:].rearrange("p t (j g) f -> p (t j) (g f)",
                                                   j=3, g=4))
                            nc.gpsimd.collective_compute(
                                "AllGather", ALU.bypass, replica_groups=GROUPS,
                                ins=[b_in.opt()], outs=[b_out.opt()])
                            for c in range(4):
                                nc.sync.dma_start(
                                    krank[c][:],
                                    b_out[12 * c:12 * c + 6, :, 0:TOK]
                                    .transpose([1, 0, 2]))
                                nc.sync.dma_start(
                                    vrank[c][:].rearrange("p t (j f) -> p (t j) f", j=3),
                                    b_out[12 * c + 6:12 * c + 12, :, :]
                                    .transpose([1, 0, 2]))

                    # attention
                    for h in range(H):
                        o = (h % 2) * 64
                        hp = h // 2
                        pts = []
                        for kc2 in range(4):
                            st = ps_st.tile([128, 2, TOK], F32, tag="st")
                            for i in range(2):
                                kc = 2 * kc2 + i
                                nc.tensor.matmul(
                                    st[:, i, :],
                                    krank[kc // 2][o:o + 64, hp,
                                                   128 * (kc % 2):128 * (kc % 2) + 128],
                                    qkv_sb[o:o + 64, hp, :],
                                    start=True, stop=True)
                            pt = pt_pool.tile([128, 2, TOK], BF16, tag="pt")
                            nc.scalar.activation(pt[:], st[:], AF.Exp)
                            nc.vector.tensor_mul(pt[:], pt[:],
                                                 mask01[:, 2 * kc2:2 * kc2 + 2, :])
                            pts.append(pt)
                        # NOTE: 65-col stationary (fused den row) corrupts on HW
                        # even though CoreSim accepts it -- use a separate den MM.
                        av = ps_a.tile([128, TOK], F32, tag="b")
                        vcol = 66 * h + 1
                        dn = ps_st.tile([1, TOK], F32, tag="dn", bufs=1)
                        for kc in range(8):
                            nc.tensor.matmul(
                                av[o:o + 64, :],
                                vrank[kc // 2][:, kc % 2, vcol:vcol + 64],
                                pts[kc // 2][:, kc % 2, :],
                                start=(kc == 0), stop=(kc == 7))
                            nc.tensor.matmul(
                                dn[:], ones_bf[:], pts[kc // 2][:, kc % 2, :],
                                start=(kc == 0), stop=(kc == 7))
                        rden = stt2.tile([1, TOK], F32, tag="rden")
                        nc.vector.reciprocal_approx_fast(rden[:], dn[:])
                        rdb = stt2.tile([1, TOK], BF16, tag="rdb")
                        nc.vector.tensor_copy(rdb[:], rden[:])
                        bc = ps_a.tile([128, TOK], F32, tag="b")
                        nc.tensor.matmul(bc[o:o + 64, :], ones_r64[:], rdb[:],
                                         start=True, stop=True)
                        bcs = tmp.tile([128, TOK], F32, tag="bcs")
                        nc.scalar.copy(bcs[o:o + 64, :], bc[o:o + 64, :])
                        nc.vector.tensor_mul(vals_fm[o:o + 64, hp, :],
                                             av[o:o + 64, :],
                                             bcs[o:o + 64, :])

                    # proj + residual (fused)
                    for m in range(NK):
                        ps = ps_a.tile([128, TOK], F32, tag="mm")
                        for k in range(NK):
                            nc.tensor.matmul(ps[:], wproj[:, k, 128 * m:128 * (m + 1)],
                                             vals_fm[:, k, :],
                                             start=(k == 0), stop=(k == NK - 1))
                        nc.vector.scalar_tensor_tensor(
                            x_fm[:, m, :], ps[:], bcol(l * PER_L + OFF_PROJB, m),
                            x_fm[:, m, :], ALU.add, ALU.add)
                        if m == 0 and l + 1 < L:
                            nc.sync.dma_start(wqkv[:], d_qkvw.ap()[l + 1])

                    gs = [bcol(l * PER_L + OFF_N2G, k) for k in range(NK)]
                    bs = [bcol(l * PER_L + OFF_N2B, k) for k in range(NK)]
                    _ln(nc, tc, pools, x_fm, gs, bs, h_bf)

                    # FFN1 (m-major streamed weights) + gelu
                    for g in range(4):
                        wg = wf1_p.tile([128, 6, NK, 128], BF16, tag="w1")
                        nc.sync.dma_start(wg[:], d_f1w.ap()[l, g])
                        for mi in range(6):
                            m = 6 * g + mi
                            ps = ps_a.tile([128, TOK], F32, tag="mm")
                            for k in range(NK):
                                nc.tensor.matmul(ps[:], wg[:, mi, k, :], h_bf[:, k, :],
                                                 start=(k == 0), stop=(k == NK - 1))
                            nc.scalar.activation(h2_sb[:, m, :], ps[:], GELU_AF,
                                                 bias=bcol(l * PER_L + OFF_F1B, m))

                    # FFN2 in two half-contractions (12-chunk weight tiles)
                    wh1 = wf2_p.tile([128, 12, D], BF16, tag="w2")
                    nc.sync.dma_start(wh1[:], d_f2w.ap()[l, 0])
                    for m in range(NK):
                        ps = ps_a.tile([128, TOK], F32, tag="mm")
                        for kk in range(12):
                            nc.tensor.matmul(ps[:], wh1[:, kk, 128 * m:128 * (m + 1)],
                                             h2_sb[:, kk, :],
                                             start=(kk == 0), stop=(kk == 11))
                        nc.vector.scalar_tensor_tensor(
                            f2acc[:, m, :], ps[:], bcol(l * PER_L + OFF_F2B, m),
                            x_fm[:, m, :], ALU.add, ALU.add)
                        if m == 0 and l + 1 < L:
                            nc.sync.dma_start(wproj[:], d_projw.ap()[l + 1])
                    wh2 = wf2_p.tile([128, 12, D], BF16, tag="w2")
                    nc.sync.dma_start(wh2[:], d_f2w.ap()[l, 1])
                    for m in range(NK):
                        ps = ps_a.tile([128, TOK], F32, tag="mm")
                        for kk in range(12):
                            nc.tensor.matmul(ps[:], wh2[:, kk, 128 * m:128 * (m + 1)],
                                             h2_sb[:, 12 + kk, :],
                                             start=(kk == 0), stop=(kk == 11))
                        nc.vector.tensor_add(x_fm[:, m, :], ps[:], f2acc[:, m, :])

                # final LN (uses body psum pools), then close body pools
                gs = [bcol(OFF_FING, k) for k in range(NK)]
                bs = [bcol(OFF_FINB, k) for k in range(NK)]
                _ln(nc, tc, pools, x_fm, gs, bs, h_bf)

                hb_in = dram.tile([NK, 128, TOK], BF16, tag="hbin")
                hb_out = dram.tile([8 * NK, 128, TOK], BF16, tag="hbout",
                                   addr_space="Shared")
                nc.sync.dma_start(hb_in[:].transpose([1, 0, 2]), h_bf[:])
                nc.gpsimd.collective_compute(
                    "AllGather", ALU.bypass, replica_groups=ALLG,
                    ins=[hb_in.opt()], outs=[hb_out.opt()])

            # lm_head: vocab-sharded over cores; all 2048 tokens per core
            with ExitStack() as lctx:
                lm_p = lctx.enter_context(tc.tile_pool(name="lm", bufs=1))
                wv_p = lctx.enter_context(tc.tile_pool(name="wv", bufs=2))
                lmo_p = lctx.enter_context(tc.tile_pool(name="lmo", bufs=3))
                ps_lm = lctx.enter_context(tc.tile_pool(name="ps_lm", bufs=6, space="PSUM"))

                h_all = lm_p.tile([128, NK, TOKALL], BF16)
                for r in range(8):
                    nc.sync.dma_start(
                        h_all[:, :, TOK * r:TOK * (r + 1)],
                        hb_out[NK * r:NK * (r + 1)].transpose([1, 0, 2]))

                for half in range(2):
                    wv = wv_p.tile([128, 4, NK, VB], BF16, tag="wv")
                    nc.sync.dma_start(wv[:], d_wemb.ap()[half])
                    for tcn in range(NTC):
                        pss = [ps_lm.tile([128, VB], F32, tag="lm", name=f"lmps{vb}")
                               for vb in range(4)]
                        for k in range(NK):
                            for vb in range(4):
                                nc.tensor.matmul(
                                    pss[vb][:],
                                    h_all[:, k, 128 * tcn:128 * (tcn + 1)],
                                    wv[:, vb, k, :],
                                    start=(k == 0), stop=(k == NK - 1))
                        outt = lmo_p.tile([128, 4 * VB], BF16, tag="ot")
                        for vb in range(4):
                            nc.vector.tensor_copy(outt[:, VB * vb:VB * (vb + 1)],
                                                  pss[vb][:])
                        nc.sync.dma_start(
                            d_out.ap()[128 * tcn:128 * (tcn + 1),
                                       2000 * half:2000 * (half + 1)],
                            outt[:])

    nc.compile()
    return nc


def _prep_inputs(W_emb, pos_emb, norm1_g, norm1_b, qkv_w, qkv_b, proj_w, proj_b,
                 norm2_g, norm2_b, ffn_w1, ffn_b1, ffn_w2, ffn_b2, fin_g, fin_b,
                 input_ids):
    bf = ml_dtypes.bfloat16
    f32 = np.float32
    isq = 1.0 / math.sqrt(HD)

    def tp(a):  # [L, out, in] -> [L, 128, NK, out] bf16 (lhsT chunks, k-major)
        a = np.asarray(a, f32)
        out_dim = a.shape[1]
        return np.ascontiguousarray(
            a.transpose(0, 2, 1).reshape(L, NK, 128, out_dim).transpose(0, 2, 1, 3)
        ).astype(bf)

    # qkv reordered to [q(all heads) | k | v] rows; q rows pre-scaled by 1/sqrt(HD)
    qkv_r = np.asarray(qkv_w, f32).reshape(L, H, 3, HD, D).transpose(0, 2, 1, 3, 4) \
        .reshape(L, 3 * D, D).copy()
    qkv_r[:, :D, :] *= isq
    qkv_b_r = np.asarray(qkv_b, f32).reshape(L, H, 3, HD).transpose(0, 2, 1, 3) \
        .reshape(L, 3 * D).copy()
    qkv_b_r[:, :D] *= isq

    # f1 m-major: [L, 4, 128, 6, NK, 128]; lhsT block (m,k) = W1[128m:.., 128k:..].T
    f1 = np.asarray(ffn_w1, f32)                       # [L, HID, D]
    f1_blk = f1.reshape(L, NM_HID, 128, NK, 128).transpose(0, 1, 3, 4, 2)  # l,m,k,in,out
    f1_m = f1_blk.reshape(L, 4, 6, NK, 128, 128).transpose(0, 1, 4, 2, 3, 5)
    f1_m = np.ascontiguousarray(f1_m).astype(bf)       # [L,4,128(in),6(m),NK,128(out)]

    # f2: [L, 2, 128, 12, D]; lhsT chunk kk = W2[:, 128kk:..].T
    f2 = np.asarray(ffn_w2, f32)                       # [L, D, HID]
    f2_t = f2.transpose(0, 2, 1).reshape(L, 2, 12, 128, D).transpose(0, 1, 3, 2, 4)
    f2_t = np.ascontiguousarray(f2_t).astype(bf)

    W_emb = np.asarray(W_emb, f32)

    # packed per-partition params
    def cols(a, nm):  # [nm*128] -> [128, nm]
        return np.asarray(a, f32).reshape(nm, 128).T

    allb = np.zeros((128, NALLB), f32)
    for l in range(L):
        base = l * PER_L
        allb[:, base + OFF_QKVB:base + OFF_QKVB + 18] = cols(qkv_b_r[l], 18)
        allb[:, base + OFF_PROJB:base + OFF_PROJB + 6] = cols(proj_b[l], 6)
        allb[:, base + OFF_F1B:base + OFF_F1B + 24] = cols(ffn_b1[l], 24)
        allb[:, base + OFF_F2B:base + OFF_F2B + 6] = cols(ffn_b2[l], 6)
        allb[:, base + OFF_N1G:base + OFF_N1G + 6] = cols(norm1_g[l], 6)
        allb[:, base + OFF_N1B:base + OFF_N1B + 6] = cols(norm1_b[l], 6)
        allb[:, base + OFF_N2G:base + OFF_N2G + 6] = cols(norm2_g[l], 6)
        allb[:, base + OFF_N2B:base + OFF_N2B + 6] = cols(norm2_b[l], 6)
    allb[:, OFF_FING:OFF_FING + 6] = cols(fin_g, 6)
    allb[:, OFF_FINB:OFF_FINB + 6] = cols(fin_b, 6)

    ids = np.asarray(input_ids).reshape(-1).astype(np.int64)
    x0 = W_emb[ids] * math.sqrt(D)
    x0 = x0 + np.asarray(pos_emb, f32)[np.tile(np.arange(S), B)]

    qkvw_t = tp(qkv_r)                # [L,128,NK,3D]
    projw_t = tp(np.asarray(proj_w, f32))

    common = {
        "qkvw": qkvw_t, "projw": projw_t, "f1w": f1_m, "f2w": f2_t,
        "allb": allb,
    }

    kg = np.arange(4 * TOK)
    in_maps = []
    for c in range(N_CORES):
        xs = np.ascontiguousarray(
            x0[TOK * c:TOK * (c + 1)].T.reshape(NK, 128, TOK).transpose(1, 0, 2)
        ).astype(f32)
        p = c % 4
        qg = p * TOK + np.arange(TOK)
        m01 = (qg[None, :] >= kg[:, None]).astype(f32)          # [1024, 256]
        m01 = np.ascontiguousarray(
            m01.reshape(8, 128, TOK).transpose(1, 0, 2)).astype(bf)
        wc = W_emb[VC * c:VC * (c + 1)]                          # [4000, 768]
        wv = wc.reshape(2, 4, VB, NK, 128).transpose(0, 4, 1, 3, 2)
        wv = np.ascontiguousarray(wv).astype(bf)                 # [2,128,4,NK,VB]
        in_maps.append({"x0": xs, "mask01": m01, "wemb": wv, **common})
    return in_maps


def kernel(**inputs):
    global LAST_RESULT, _NC_CACHE
    in_maps = _prep_inputs(**inputs)
    if _NC_CACHE is None:
        _NC_CACHE = build_nc()
    res = run_bass_kernel_spmd(_NC_CACHE, in_maps, list(range(N_CORES)),
                               trace=TRACE)
    LAST_RESULT = res
    logits = np.concatenate(
        [np.asarray(res.results[c]["logits"]).astype(np.float32)
         for c in range(N_CORES)], axis=1)
    return logits.reshape(B, S, V)
